# revision 1
# baseline (speedup 1.0000x reference)
"""Contrastive (InfoNCE-style) loss kernel for Trainium2, SPMD over 8 NeuronCores.

Math: emb [2, N, D] -> v1 = l2norm(emb[0]), v2 = l2norm(emb[1])
  loss = -sum_i [ (v1_i . v2_i)/T - log sum_j exp((v1_i . v2_j)/T) ]

Sharding: rows of v1 across 8 cores (2048 each); every core holds all of v2.
Each core computes its [2048, 16384] similarity block in 128x2048 PSUM tiles
(bf16 matmul), applies exp with a fused row-sum on the Scalar engine
(accum_out), and returns per-row partial results. The host combines in f64.

Key tricks:
  - v1's normalization is folded into the exp via ACT's per-partition scale
    (scale = 1/(T*|u_i|)), so v1 stays raw in the matmul.
  - rsqrt for v2's norm is computed as exp(-0.5*ln(x)) so the only ACT
    functions used are Ln/Exp (one table set -> no table-load thrashing).
  - ttl is permutation-invariant in j, so v2 is loaded with fully-contiguous
    64KB-per-partition DMA descriptors and block-transposed in whatever
    column order that produces.
"""

import math
from contextlib import ExitStack

import numpy as np

import concourse.bass as bass
import concourse.bacc as bacc
import concourse.mybir as mybir
from concourse.masks import make_identity
from concourse.tile import TileContext

P = 128
D = 128
TEMP = 0.2
N_TOTAL = 16384
N_CORES = 8
M_CORE = N_TOTAL // N_CORES  # 2048 rows of v1 per core
GROUP = 16                   # w row-blocks per group -> 2048 sim columns
MM_N = 512                   # moving-operand columns per matmul

f32 = mybir.dt.float32
bf16 = mybir.dt.bfloat16


def build_kernel(n_total: int = N_TOTAL, m_core: int = M_CORE,
                 repeat: int = 1, dummy_w: bool = False,
                 repeat_full: bool = False) -> bass.Bass:
    s_blocks = m_core // P        # u row-blocks (16)
    r_blocks = n_total // P       # w row-blocks (128)
    assert r_blocks % GROUP == 0
    n_groups = r_blocks // GROUP  # 8
    gw = GROUP * D                # sim columns per group (2048)
    assert s_blocks <= GROUP

    nc = bacc.Bacc()
    u_in = nc.declare_dram_parameter("u", [m_core, D], f32, isOutput=False)
    wown_in = nc.declare_dram_parameter("wown", [m_core, D], f32, isOutput=False)
    w_in = nc.declare_dram_parameter(
        "w", [P, D] if dummy_w else [n_total, D], f32, isOutput=False)
    ttl_out = nc.declare_dram_parameter("ttl", [P, s_blocks], f32, isOutput=True)
    draw_out = nc.declare_dram_parameter("draw", [P, s_blocks], f32, isOutput=True)
    nsqu_out = nc.declare_dram_parameter("nsqu", [P, s_blocks], f32, isOutput=True)
    nsqw_out = nc.declare_dram_parameter("nsqw", [P, s_blocks], f32, isOutput=True)

    # Partition p holds a contiguous slab of rows: u4[p, s*D+d] = u[p*s_blocks+s, d]
    u_ap = u_in[:].rearrange("(p s) d -> p (s d)", p=P)
    wown_ap = wown_in[:].rearrange("(p s) d -> p (s d)", p=P)
    w_ap = None if dummy_w else w_in[:].rearrange("(p r) d -> p (r d)", p=P)

    mult = mybir.AluOpType.mult
    add = mybir.AluOpType.add
    Ln = mybir.ActivationFunctionType.Ln
    Exp = mybir.ActivationFunctionType.Exp

    with TileContext(nc) as tc, ExitStack() as ctx:
        consts = ctx.enter_context(tc.tile_pool(name="consts", bufs=1))
        big = ctx.enter_context(tc.tile_pool(name="big", bufs=1))
        small = ctx.enter_context(tc.tile_pool(name="small", bufs=1))
        sqp = ctx.enter_context(tc.tile_pool(name="sqp", bufs=2))
        esp = ctx.enter_context(tc.tile_pool(name="esp", bufs=2))
        psum = ctx.enter_context(tc.tile_pool(name="psum", bufs=2, space="PSUM"))

        identity = consts.tile([P, P], f32)
        make_identity(nc, identity)
        neg_ln_t = consts.tile([P, 1], f32)
        nc.vector.memset(neg_ln_t, -math.log(TEMP))

        # PE observes the gpsimd (identity) semaphore here, so later real
        # transposes carry a single sync wait (Matmult allows only one).
        warm = psum.tile([P, gw], f32, tag="S")
        nc.tensor.transpose(warm[:, :P], identity, identity)

        A = big.tile([P, n_total], f32)      # w, slab layout (later normalized in place)
        w_t = big.tile([P, n_total], bf16)   # normalized w, transposed (cols permuted)
        u4 = big.tile([P, m_core], f32)
        u_t = big.tile([P, m_core], bf16)
        wown = big.tile([P, m_core], f32)

        nsqu = small.tile([P, s_blocks], f32)
        nsqw = small.tile([P, s_blocks], f32)
        draw = small.tile([P, s_blocks], f32)
        ru = small.tile([P, s_blocks], f32)
        lnu = small.tile([P, s_blocks], f32)
        nsqA = small.tile([P, r_blocks], f32)
        rwA = small.tile([P, r_blocks], f32)
        lnA = small.tile([P, r_blocks], f32)
        tacc = small.tile([P, s_blocks * n_groups], f32)
        ttl = small.tile([P, s_blocks], f32)

        dma = nc.sync

        # ---------------- u / wown prep ----------------
        dma.dma_start(out=u4, in_=u_ap)
        dma.dma_start(out=wown, in_=wown_ap)
        for s in range(s_blocks):
            blk = slice(s * D, (s + 1) * D)
            sq = sqp.tile([P, D], f32, tag="sq")
            nc.vector.tensor_mul(out=sq, in0=u4[:, blk], in1=u4[:, blk])
            nc.vector.reduce_sum(out=nsqu[:, s : s + 1], in_=sq,
                                 axis=mybir.AxisListType.X)
            sq = sqp.tile([P, D], f32, tag="sq")
            nc.vector.tensor_mul(out=sq, in0=wown[:, blk], in1=wown[:, blk])
            nc.vector.reduce_sum(out=nsqw[:, s : s + 1], in_=sq,
                                 axis=mybir.AxisListType.X)
            sq = sqp.tile([P, D], f32, tag="sq")
            nc.vector.tensor_mul(out=sq, in0=u4[:, blk], in1=wown[:, blk])
            nc.vector.reduce_sum(out=draw[:, s : s + 1], in_=sq,
                                 axis=mybir.AxisListType.X)
        # ru = 1/(T*|u|) = exp(-0.5*ln(nsqu) - ln(T))
        nc.scalar.activation(out=lnu, in_=nsqu, func=Ln)
        nc.scalar.activation(out=ru, in_=lnu, func=Exp, scale=-0.5, bias=neg_ln_t)
        dma.dma_start(out=nsqu_out[:], in_=nsqu)
        dma.dma_start(out=nsqw_out[:], in_=nsqw)
        dma.dma_start(out=draw_out[:], in_=draw)

        # u_t: transpose u (f32 -> PSUM), copy back casting to bf16
        pst = psum.tile([P, gw], f32, tag="S")
        for s in range(s_blocks):
            blk = slice(s * D, (s + 1) * D)
            nc.tensor.transpose(pst[:, blk], u4[:, blk], identity)
        nc.vector.tensor_copy(out=u_t, in_=pst[:, : s_blocks * D])

        def main_block(g):
            for m in range(s_blocks):
                ps = psum.tile([P, gw], f32, tag="S")
                for k4 in range(gw // MM_N):
                    nsl = slice(k4 * MM_N, (k4 + 1) * MM_N)
                    nc.tensor.matmul(
                        ps[:, nsl],
                        u_t[:, m * D : (m + 1) * D],
                        w_t[:, g * gw + k4 * MM_N : g * gw + (k4 + 1) * MM_N],
                        start=True, stop=True)
                es = esp.tile([P, gw], bf16, tag="es")
                nc.scalar.activation(
                    out=es, in_=ps, func=Exp, scale=ru[:, m : m + 1],
                    accum_out=tacc[:, m * n_groups + g : m * n_groups + g + 1])
                # WAR-ordered after the ACT read: makes DVE the last accessor
                # of the PSUM slot so the next matmul's slot wait merges with
                # its other DVE deps into one sync wait.
                nc.vector.memset(ps[:, :1], 0.0)

        def w_pass(with_main):
            if dummy_w:
                nc.vector.memset(A, 0.01)
            for g in range(n_groups):
                gs = slice(g * gw, (g + 1) * gw)
                if not dummy_w:
                    dma.dma_start(out=A[:, gs], in_=w_ap[:, gs])
                for k in range(GROUP):
                    r = g * GROUP + k
                    blk = slice(r * D, (r + 1) * D)
                    sq = sqp.tile([P, D], f32, tag="sq")
                    nc.vector.tensor_mul(out=sq, in0=A[:, blk], in1=A[:, blk])
                    nc.vector.reduce_sum(out=nsqA[:, r : r + 1], in_=sq,
                                         axis=mybir.AxisListType.X)
                gb = slice(g * GROUP, (g + 1) * GROUP)
                nc.scalar.activation(out=lnA[:, gb], in_=nsqA[:, gb], func=Ln)
                nc.scalar.activation(out=rwA[:, gb], in_=lnA[:, gb], func=Exp,
                                     scale=-0.5)
                for k in range(GROUP):
                    r = g * GROUP + k
                    blk = slice(r * D, (r + 1) * D)
                    nc.vector.tensor_scalar_mul(
                        out=A[:, blk], in0=A[:, blk], scalar1=rwA[:, r : r + 1])
                pg = psum.tile([P, gw], f32, tag="S")
                for k in range(GROUP):
                    r = g * GROUP + k
                    nc.tensor.transpose(
                        pg[:, k * D : (k + 1) * D], A[:, r * D : (r + 1) * D],
                        identity)
                nc.vector.tensor_copy(out=w_t[:, gs], in_=pg)
                if with_main:
                    main_block(g)

        # ------------- w prep + main loop, one 2048-col group at a time -------
        if repeat_full:
            for _rep in range(repeat):
                w_pass(True)
        else:
            w_pass(repeat >= 1)
            for _rep in range(1, repeat):
                for g in range(n_groups):
                    main_block(g)

        if repeat >= 1:
            for m in range(s_blocks):
                nc.vector.reduce_sum(
                    out=ttl[:, m : m + 1],
                    in_=tacc[:, m * n_groups : (m + 1) * n_groups],
                    axis=mybir.AxisListType.X)
        else:
            nc.vector.memset(ttl, 1.0)
        dma.dma_start(out=ttl_out[:], in_=ttl)

    nc.compile()
    return nc


_NC_CACHE: dict = {}


def _get_nc(n_total: int, m_core: int) -> bass.Bass:
    key = (n_total, m_core)
    if key not in _NC_CACHE:
        _NC_CACHE[key] = build_kernel(n_total, m_core)
    return _NC_CACHE[key]


def _combine(results: list[dict], temp: float = TEMP) -> np.float32:
    total = 0.0
    for r in results:
        ttl = r["ttl"].astype(np.float64).reshape(-1)
        draw = r["draw"].astype(np.float64).reshape(-1)
        nsqu = r["nsqu"].astype(np.float64).reshape(-1)
        nsqw = r["nsqw"].astype(np.float64).reshape(-1)
        norms = np.maximum(np.sqrt(nsqu), 1e-12) * np.maximum(np.sqrt(nsqw), 1e-12)
        total += np.sum(np.log(ttl) - draw / (temp * norms))
    return np.float32(total)


def kernel(emb: np.ndarray) -> np.ndarray:
    from concourse.bass_utils import run_bass_kernel_spmd

    emb = np.ascontiguousarray(np.asarray(emb, dtype=np.float32))
    assert emb.shape == (2, N_TOTAL, D), emb.shape
    nc = _get_nc(N_TOTAL, M_CORE)
    in_maps = []
    for c in range(N_CORES):
        sl = slice(c * M_CORE, (c + 1) * M_CORE)
        in_maps.append({
            "u": np.ascontiguousarray(emb[0, sl]),
            "wown": np.ascontiguousarray(emb[1, sl]),
            "w": np.ascontiguousarray(emb[1]),
        })
    res = run_bass_kernel_spmd(nc, in_maps, core_ids=list(range(N_CORES)))
    return np.array(_combine(res.results), dtype=np.float32)

